# revision 1
# baseline (speedup 1.0000x reference)
"""Trainium2 kernel for nn_InterpolatorMaskArgs (embedding_lookup, memory regime).

reference computes:  ind = floor((x[0]-X0)/DX);  res = sum(roll(mask, ind) * yOrig)
i.e. a full O(N) dot product between yOrig and the rolled mask, with an
out-of-range guard on x.

Strategy (matches the sharding hint):
  - 1-D shard yOrig along N across the 8 cores (contiguous 2M-element shards).
  - The roll is resolved at shard time: core c receives the slice of the
    rolled mask aligned with its yOrig shard, i.e. mask[(c*S - ind) mod N ...]
    (mod-N wraparound == the halo exchange, done while scattering inputs).
  - Host packs each core's y-shard and mask-shard into one [P, 2, F] input so
    every SBUF tile arrives via a single DMA (one DMA-lane semaphore per
    consumer; the TensorTensor ISA slot only fits one wait).
  - Per tile: VectorE multiplies in place, ScalarE reduces the product to 128
    per-partition partials (activation-Copy accum_out). Both engines stay
    well under the ~45us/core DMA roofline (16 MiB @ ~358 GB/s).
  - The final all-reduce of per-shard partials is done on the host over the
    8*128*NT partials (a few KB), followed by the out-of-range predicate.
"""

import numpy as np

import concourse.bass as bass
import concourse.mybir as mybir
from concourse.bass_utils import run_bass_kernel_spmd

# Grid constants (must match the problem's reference.py)
N = 16777216
X0 = 0.0
DX = 1.0
XMAX = X0 + (N - 1) * DX

NCORES = 8
P = 128                 # SBUF partitions
S = N // NCORES         # 2,097,152 elements per core
F = S // P              # 16,384 free-dim elements per partition
T = 2048                # tile free width (128 x 2 x 2048 f32 = 2 MiB per DMA)
NT = F // T             # tiles per shard

_CACHED_NC = None
NB = 3                  # SBUF buffer slots (triple buffering)


def _build_nc():
    """Raw Bass (not Tile): this walrus build rejects instructions carrying
    more than ~1 inline semaphore wait ("Too many sync wait commands"), so
    all cross-engine sync uses standalone wait_ge instructions."""
    nc = bass.Bass(trn_type="TRN2")
    ym = nc.dram_tensor("ym", [P, 2, F], mybir.dt.float32, kind="ExternalInput")
    out = nc.dram_tensor("out", [P, NT], mybir.dt.float32, kind="ExternalOutput")

    f32 = mybir.dt.float32
    with (
        nc.Block() as block,
        nc.semaphore("dma0") as d0,
        nc.semaphore("dma1") as d1,
        nc.semaphore("dma2") as d2,
        nc.semaphore("mul_sem") as mul_sem,
        nc.semaphore("act_sem") as act_sem,
        nc.semaphore("out_sem") as out_sem,
        nc.sbuf_tensor("ct0", [P, 2, T], f32) as ct0,
        nc.sbuf_tensor("ct1", [P, 2, T], f32) as ct1,
        nc.sbuf_tensor("ct2", [P, 2, T], f32) as ct2,
        nc.sbuf_tensor("acc", [P, NT], f32) as acc,
    ):
        dsems = [d0, d1, d2]
        cts = [ct0, ct1, ct2]

        @block.sync
        def _(sync):
            for i in range(NT):
                b = i % NB
                if i >= NB:
                    # slot reuse: wait until act(i-NB) is done with it
                    sync.wait_ge(act_sem, i - NB + 1)
                sync.dma_start(
                    out=cts[b][:], in_=ym[:, :, i * T:(i + 1) * T]
                ).then_inc(dsems[b], 16)
            sync.wait_ge(act_sem, NT)
            sync.dma_start(out=out[:], in_=acc[:]).then_inc(out_sem, 16)
            sync.wait_ge(out_sem, 16)

        @block.vector
        def _(vector):
            for i in range(NT):
                b = i % NB
                vector.wait_ge(dsems[b], 16 * (i // NB + 1))
                # in-place product into the y half
                nc.vector.tensor_mul(
                    out=cts[b][:, 0, :], in0=cts[b][:, 0, :], in1=cts[b][:, 1, :]
                ).then_inc(mul_sem, 1)

        @block.scalar
        def _(scalar):
            for i in range(NT):
                b = i % NB
                scalar.wait_ge(mul_sem, i + 1)
                # acc[:, i] = per-partition free-dim sum of the product;
                # the mandatory full-width copy lands in the (dead) m half
                nc.scalar.activation(
                    out=cts[b][:, 1, :],
                    in_=cts[b][:, 0, :],
                    func=mybir.ActivationFunctionType.Copy,
                    accum_out=acc[:, i:i + 1],
                ).then_inc(act_sem, 1)

    return nc


def _get_nc():
    global _CACHED_NC
    if _CACHED_NC is None:
        _CACHED_NC = _build_nc()
    return _CACHED_NC


def kernel(x, yOrig, mask):
    x = np.asarray(x)
    yOrig = np.ascontiguousarray(np.asarray(yOrig, dtype=np.float32))
    mask = np.ascontiguousarray(np.asarray(mask, dtype=np.float32))

    xs = float(x.reshape(-1)[0])
    ind = int(np.floor((xs - X0) / DX))
    shift = ind % N

    # rolled[i] = mask[(i - ind) mod N]  (== np.roll(mask, ind))
    if shift == 0:
        rolled = mask
    else:
        rolled = np.concatenate([mask[N - shift:], mask[:N - shift]])

    in_maps = []
    for c in range(NCORES):
        ymc = np.empty((P, 2, F), dtype=np.float32)
        ymc[:, 0, :] = yOrig[c * S:(c + 1) * S].reshape(P, F)
        ymc[:, 1, :] = rolled[c * S:(c + 1) * S].reshape(P, F)
        in_maps.append({"ym": ymc})

    res = run_bass_kernel_spmd(_get_nc(), in_maps, core_ids=list(range(NCORES)))

    partials = np.concatenate([r["out"].reshape(-1) for r in res.results])
    total = np.float32(partials.sum(dtype=np.float32))

    if xs >= XMAX or xs < X0:
        total = np.float32(0.0)

    # Stash for test harnesses that want profiling info.
    kernel.last_results = res
    return np.asarray(total, dtype=np.float32)



# revision 3
# speedup vs baseline: 1.5196x; 1.5196x over previous
"""Trainium2 kernel for nn_InterpolatorMaskArgs (embedding_lookup, memory regime).

reference computes:  ind = floor((x[0]-X0)/DX);  res = sum(roll(mask, ind) * yOrig)
i.e. a full O(N) dot product between yOrig and the rolled mask, with an
out-of-range guard on x.

Strategy (matches the sharding hint):
  - 1-D shard yOrig along N across the 8 cores (contiguous 2M-element shards).
  - The roll is resolved at shard time: core c receives the slice of the
    rolled mask aligned with its yOrig shard (mod-N wraparound == the halo
    exchange, done while scattering inputs).
  - Both streams are sent in bf16 (the 2e-2 rel-err budget dwarfs bf16's
    ~4e-3 rounding), halving HBM traffic: 8.4 MiB/core -> ~23.4 us at the
    ~358 GB/s per-core HBM limit.
  - Host packs each core's y-shard and mask-shard into one [P, 2, F] bf16
    input so every SBUF tile arrives via a single DMA.
  - The whole shard is SBUF-resident (64 KiB/partition), so the DMA stream
    never throttles on compute; tiles pipeline into VectorE.
  - Per tile: one DVE tensor_tensor_reduce (product + free-dim sum into a
    fp32 per-partition accumulator column). No ScalarE pass, no act tables.
  - The final all-reduce of per-shard partials is done on the host over the
    8*128*NT partials (a few KB), followed by the out-of-range predicate.
"""

import numpy as np
import ml_dtypes

import concourse.bass as bass
import concourse.mybir as mybir
from concourse.bass_utils import run_bass_kernel_spmd

# Grid constants (must match the problem's reference.py)
N = 16777216
X0 = 0.0
DX = 1.0
XMAX = X0 + (N - 1) * DX

NCORES = 8
P = 128                 # SBUF partitions
S = N // NCORES         # 2,097,152 elements per core
F = S // P              # 16,384 free-dim elements per partition
NT = 8                  # tiles per shard
T = F // NT             # 2048 bf16 elems -> [128, 2, 2048] = 1 MiB per DMA
ND = 4                  # rotating DMA-completion semaphores

BF16 = ml_dtypes.bfloat16

_CACHED_NC = None


def _build_nc():
    """Raw Bass (not Tile): this walrus build rejects instructions carrying
    more than ~1 inline semaphore wait, so cross-engine sync uses standalone
    wait_ge instructions."""
    nc = bass.Bass(trn_type="TRN2")
    ym = nc.dram_tensor("ym", [P, 2, F], mybir.dt.bfloat16, kind="ExternalInput")
    out = nc.dram_tensor("out", [P, NT], mybir.dt.float32, kind="ExternalOutput")

    f32 = mybir.dt.float32
    bf16 = mybir.dt.bfloat16
    with (
        nc.Block(no_gpsimd_drain=True) as block,
        nc.semaphore("d0") as d0,
        nc.semaphore("d1") as d1,
        nc.semaphore("d2") as d2,
        nc.semaphore("d3") as d3,
        nc.semaphore("vec_sem") as vec_sem,
        nc.semaphore("out_sem") as out_sem,
        nc.sbuf_tensor("yb", [P, 2, F], bf16) as yb,
        nc.sbuf_tensor("acc", [P, NT], f32) as acc,
    ):
        dsems = [d0, d1, d2, d3]

        @block.sync
        def _(sync):
            # Whole shard is resident: the DMA stream has no compute
            # feedback, so it runs gapless at the HBM rate.
            for i in range(NT):
                sync.dma_start(
                    out=yb[:, :, i * T:(i + 1) * T],
                    in_=ym[:, :, i * T:(i + 1) * T],
                ).then_inc(dsems[i % ND], 16)
            sync.wait_ge(vec_sem, NT)
            sync.dma_start(out=out[:], in_=acc[:]).then_inc(out_sem, 16)
            sync.wait_ge(out_sem, 16)

        @block.vector
        def _(vector):
            for i in range(NT):
                vector.wait_ge(dsems[i % ND], 16 * (i // ND + 1))
                # acc[:, i] = sum over free dim of (y*1)*m; product scratch
                # is written in place over the (now dead) y half.
                nc.vector.scalar_tensor_tensor(
                    out=yb[:, 0, i * T:(i + 1) * T],
                    in0=yb[:, 0, i * T:(i + 1) * T],
                    scalar=1.0,
                    in1=yb[:, 1, i * T:(i + 1) * T],
                    op0=mybir.AluOpType.mult,
                    op1=mybir.AluOpType.mult,
                    accum_out=acc[:, i:i + 1],
                ).then_inc(vec_sem, 1)

    return nc


def _get_nc():
    global _CACHED_NC
    if _CACHED_NC is None:
        _CACHED_NC = _build_nc()
    return _CACHED_NC


def kernel(x, yOrig, mask):
    x = np.asarray(x)
    yOrig = np.ascontiguousarray(np.asarray(yOrig, dtype=np.float32))
    mask = np.ascontiguousarray(np.asarray(mask, dtype=np.float32))

    xs = float(x.reshape(-1)[0])
    ind = int(np.floor((xs - X0) / DX))
    shift = ind % N

    in_maps = []
    for c in range(NCORES):
        ymc = np.empty((P, 2, F), dtype=BF16)
        ymc[:, 0, :] = yOrig[c * S:(c + 1) * S].reshape(P, F)
        # rolled[j] = mask[(j - shift) mod N] for j in [c*S, (c+1)*S)
        start = (c * S - shift) % N
        if start + S <= N:
            mc = mask[start:start + S]
        else:
            mc = np.concatenate([mask[start:], mask[:start + S - N]])
        ymc[:, 1, :] = mc.reshape(P, F)
        in_maps.append({"ym": ymc})

    res = run_bass_kernel_spmd(_get_nc(), in_maps, core_ids=list(range(NCORES)))

    partials = np.concatenate([r["out"].reshape(-1) for r in res.results])
    total = np.float32(partials.sum(dtype=np.float32))

    if xs >= XMAX or xs < X0:
        total = np.float32(0.0)

    # Stash for test harnesses that want profiling info.
    kernel.last_results = res
    return np.asarray(total, dtype=np.float32)
